# revision 6
# baseline (speedup 1.0000x reference)
"""GNN message passing + aggregation + linear projection on 8 TRN2 NeuronCores.

Reference computation:
    msgs = (features[pair_i] + features[pair_j]) * pair_w[:, None]   # [P, 128]
    agg  = segment_sum(msgs, node_idx, 50000)                        # [N, 128]
    out  = agg @ weight + bias                                       # [N, 128]

Strategy (edge parallelism, no collectives):
  - node space padded to 50176 = 8 cores x 49 windows x 128 nodes.
  - node_idx is sorted, so pairs are sharded by center-node window; each core
    owns a contiguous disjoint node range => no cross-core reduction needed.
  - per (window, class) groups: class = (i%2, j%2) parity; pairs are gathered
    with dma_gather (int16 indices) from a [25000, 256] bf16 parity view of
    the feature table (node n -> row n>>1, column half n&1).
  - per 128-pair chunk: DVE builds E' [c,n] = (iota_n == rel_c) * w_c (one-hot
    times weight); PE accumulates psum[n,d] += E'.T @ gathered_i + E'.T @
    gathered_j over the window; then transpose + GEMM with weight + bias.
"""
import numpy as np
import ml_dtypes
import contextlib

import concourse.bass as bass
import concourse.mybir as mybir
from concourse import bacc
from concourse.bass_utils import run_bass_kernel_spmd
from concourse.library_config import mlp

P = 128
N_NODES = 50000
NPAD = 50176            # 392 windows * 128
NWIN = 392              # global windows
NCORES = 8
WPC = NWIN // NCORES    # 49 windows per core
D = 128
NI_MAX = 4096           # idxs per dma_gather instruction
CHUNKS_PER_INSTR = NI_MAX // P  # 32


def _wrap16_block(v):
    """[ni] int16 -> [128, ni//16] wrapped-16 replicated layout."""
    ni = v.shape[0]
    a = v.reshape(ni // 16, 16).T          # [16, ni//16]
    return np.tile(a, (8, 1))              # [128, ni//16]


def _plan(node_idx, pair_i, pair_j, pair_w):
    n = node_idx.shape[0]
    win = (node_idx // P).astype(np.int64)
    cls = ((pair_i & 1) * 2 + (pair_j & 1)).astype(np.int64)
    key = win * 4 + cls
    perm = np.argsort(key, kind="stable")
    skey = key[perm]
    counts = np.bincount(key, minlength=NWIN * 4).reshape(NWIN, 4)
    K = np.maximum(1, np.ceil(counts.max(axis=0) / P).astype(np.int64))  # per class
    SUMK = int(K.sum())
    CAPC = WPC * SUMK * P  # slots per core
    # class-major slot bases within a core
    base_q = np.zeros(4, dtype=np.int64)
    for q in range(1, 4):
        base_q[q] = base_q[q - 1] + WPC * K[q - 1] * P

    group_starts = np.searchsorted(skey, np.arange(NWIN * 4), side="left")
    rank = np.arange(n) - group_starts[skey]
    w_s = win[perm]
    q_s = cls[perm]
    core_s = w_s // WPC
    slot_in_core = base_q[q_s] + (w_s % WPC) * K[q_s] * P + rank
    gslot = core_s * CAPC + slot_in_core

    tot = NCORES * CAPC
    gi16 = np.zeros(tot, dtype=np.int16)
    gj16 = np.zeros(tot, dtype=np.int16)
    icol = np.zeros(tot, dtype=np.int8)   # parity of i (for sanity only)
    rel = np.full(tot, -1.0, dtype=np.float32)
    wv = np.zeros(tot, dtype=np.float32)

    gi16[gslot] = (pair_i[perm] >> 1).astype(np.int16)
    gj16[gslot] = (pair_j[perm] >> 1).astype(np.int16)
    icol[gslot] = (pair_i[perm] & 1).astype(np.int8)
    rel[gslot] = (node_idx[perm] - w_s * P).astype(np.float32)
    wv[gslot] = pair_w[perm].astype(np.float32)

    # dummy slots in class q must gather from the right parity column; idx 0 is
    # always valid for either column, so nothing else needed.
    plan = dict(K=[int(k) for k in K], SUMK=SUMK, CAPC=CAPC, base_q=base_q)
    return plan, gi16.reshape(NCORES, CAPC), gj16.reshape(NCORES, CAPC), \
        rel.reshape(NCORES, CAPC), wv.reshape(NCORES, CAPC)


def _instr_list(Lq):
    """Split a class stream of Lq idxs into dma_gather instruction sizes."""
    out = []
    off = 0
    while off < Lq:
        ni = min(NI_MAX, Lq - off)
        out.append((off, ni))
        off += ni
    return out


def _build(plan):
    K = plan["K"]
    SUMK = plan["SUMK"]
    base_q = plan["base_q"]
    # per class stream length (both sides identical)
    Lq = [WPC * K[q] * P for q in range(4)]
    instrs = [_instr_list(Lq[q]) for q in range(4)]   # same for both sides
    n_chunks = WPC * SUMK                              # per core chunk count
    # idx dram col offsets per (q, s, j)
    idx_cols = [[[0] * len(instrs[q]) for _ in range(2)] for q in range(4)]
    off = 0
    for q in range(4):
        for s in range(2):
            for j, (soff, ni) in enumerate(instrs[q]):
                idx_cols[q][s][j] = off
                off += ni // 16
    IDXCOLS = off

    nc = bacc.Bacc()
    dt = mybir.dt
    tab = nc.declare_dram_parameter("tab", [N_NODES // 2, 256], dt.bfloat16, isOutput=False)
    idx_d = nc.declare_dram_parameter("idx", [P, IDXCOLS], dt.int16, isOutput=False)
    rel_d = nc.declare_dram_parameter("rel", [P, n_chunks], dt.float32, isOutput=False)
    wv_d = nc.declare_dram_parameter("wv", [P, n_chunks], dt.float32, isOutput=False)
    iota_d = nc.declare_dram_parameter("iota", [P, P], dt.float32, isOutput=False)
    ident_d = nc.declare_dram_parameter("ident", [P, P], dt.float32, isOutput=False)
    wt_d = nc.declare_dram_parameter("wt", [P, P], dt.float32, isOutput=False)
    bias_d = nc.declare_dram_parameter("biasb", [P, P], dt.float32, isOutput=False)
    out_d = nc.declare_dram_parameter("out", [WPC, P, P], dt.float32, isOutput=True)

    NEBUF = 6

    with (
        nc.Block() as block,
        contextlib.ExitStack() as st,
    ):
        sem = nc.semaphore
        gsem = [[st.enter_context(sem(f"g{q}{s}")) for s in range(2)] for q in range(4)]
        isem = [[st.enter_context(sem(f"i{q}{s}")) for s in range(2)] for q in range(4)]
        csem = st.enter_context(sem("consts"))
        evsem = st.enter_context(sem("ev"))
        pe_c = st.enter_context(sem("pe_c"))
        pe_t = st.enter_context(sem("pe_t"))
        pe_g = st.enter_context(sem("pe_g"))
        act_a = st.enter_context(sem("act_a"))
        act_b = st.enter_context(sem("act_b"))
        dv_o = st.enter_context(sem("dv_o"))
        osem = st.enter_context(sem("osem"))

        sb = lambda name, shape, d_: st.enter_context(nc.sbuf_tensor(name, shape, d_))
        dst = [[[sb(f"dst{q}{s}{b}", [P, CHUNKS_PER_INSTR, D], dt.bfloat16)
                 for b in range(2)] for s in range(2)] for q in range(4)]
        idx_t = [[[sb(f"idx{q}{s}{b}", [P, NI_MAX // 16], dt.int16)
                   for b in range(2)] for s in range(2)] for q in range(4)]
        rel_t = sb("rel_t", [P, n_chunks], dt.float32)
        wv_t = sb("wv_t", [P, n_chunks], dt.float32)
        iota_t = sb("iota_t", [P, P], dt.float32)
        ident_t = sb("ident_t", [P, P], dt.float32)
        wt_t = sb("wt_t", [P, P], dt.float32)
        bias_t = sb("bias_t", [P, P], dt.float32)
        ep_t = [sb(f"ep{b}", [P, P], dt.bfloat16) for b in range(NEBUF)]
        agg_t = [sb(f"agg{b}", [P, P], dt.float32) for b in range(2)]
        aggT_t = [sb(f"aggT{b}", [P, P], dt.float32) for b in range(2)]
        out_t = [sb(f"out{b}", [P, P], dt.float32) for b in range(2)]

        ps = lambda name: st.enter_context(nc.psum_tensor(name, [P, 512], dt.float32))
        ps_agg = [ps(f"psagg{b}") for b in range(2)]
        ps_tr = [ps(f"pstr{b}") for b in range(2)]
        ps_gm = [ps(f"psgm{b}") for b in range(2)]

        # round-robin gather issue order shared by sync (idx loads) and gpsimd
        issue = []  # (q, s, j, dram_coloff, ni)
        maxj = max(len(instrs[q]) for q in range(4))
        for j in range(maxj):
            for q in range(4):
                if j < len(instrs[q]):
                    for s in range(2):
                        issue.append((q, s, j, idx_cols[q][s][j], instrs[q][j][1]))

        # window consumed by end of window w -> pe_w >= w+1
        # gather (q,s,j) with j>=2 may overwrite dst buffer of instr j-2 whose
        # last chunk is in window wlast = (min((j-1)*32, Lq/128)-1)//K[q]
        def wlast(q, j):
            last_chunk = min((j - 1) * CHUNKS_PER_INSTR, Lq[q] // P) - 1
            return last_chunk // K[q]

        @block.sync
        def _(eng: bass.BassEngine):
            for ii, (dst_sb, src_d) in enumerate([
                (rel_t, rel_d), (wv_t, wv_d), (iota_t, iota_d),
                (ident_t, ident_d), (wt_t, wt_d), (bias_t, bias_d),
            ]):
                eng.dma_start(dst_sb[:], src_d[:]).then_inc(csem, 16)
                eng.wait_ge(csem, 16 * (ii + 1))
            for (q, s, j, coloff, ni) in issue:
                if j >= 1:
                    eng.wait_ge(isem[q][s], 16 * j)
                if j >= 2:
                    eng.wait_ge(gsem[q][s], 16 * (j - 1))
                eng.dma_start(
                    idx_t[q][s][j % 2][:, : ni // 16],
                    idx_d[:, coloff : coloff + ni // 16],
                ).then_inc(isem[q][s], 16)

        @block.gpsimd
        def _(gp: bass.BassGpSimd):
            gp.load_library(mlp)
            for (q, s, j, coloff, ni) in issue:
                gp.wait_ge(isem[q][s], 16 * (j + 1))
                if j >= 1:
                    gp.wait_ge(gsem[q][s], 16 * j)
                if j >= 2:
                    gp.wait_ge(pe_c, (wlast(q, j) + 1) * SUMK)
                col = (q >> 1) if s == 0 else (q & 1)
                gp.dma_gather(
                    dst[q][s][j % 2][:, : ni // P, :],
                    tab[:, col * 128 : col * 128 + 128],
                    idx_t[q][s][j % 2][:, : ni // 16],
                    ni, ni, 128,
                    elem_step=256,
                    single_packet=False,
                ).then_inc(gsem[q][s], 16)

        @block.vector
        def _(dv: bass.BassVectorEngine):
            dv.wait_ge(csem, 96)
            ci = 0
            for w in range(WPC):
                for q in range(4):
                    for k in range(K[q]):
                        cid = base_q[q] // P + w * K[q] + k  # storage col
                        if ci >= NEBUF:
                            dv.wait_ge(pe_c, ci - NEBUF + 1)
                        dv.tensor_scalar(
                            out=ep_t[ci % NEBUF][:],
                            in0=iota_t[:],
                            scalar1=rel_t[:, cid : cid + 1],
                            scalar2=wv_t[:, cid : cid + 1],
                            op0=mybir.AluOpType.is_equal,
                            op1=mybir.AluOpType.mult,
                        ).then_inc(evsem, 1)
                        ci += 1
                # bias add for window w-1 (after its GEMM)
                if w >= 1:
                    dv.wait_ge(pe_g, w)
                    if w >= 3:
                        dv.wait_ge(osem, 16 * (w - 2))
                    dv.tensor_add(
                        out=out_t[(w - 1) % 2][:],
                        in0=ps_gm[(w - 1) % 2][:, 0:P],
                        in1=bias_t[:],
                    ).then_inc(dv_o, 1)
            dv.wait_ge(osem, 16 * (WPC - 2))
            dv.wait_ge(pe_g, WPC)
            dv.tensor_add(
                out=out_t[(WPC - 1) % 2][:],
                in0=ps_gm[(WPC - 1) % 2][:, 0:P],
                in1=bias_t[:],
            ).then_inc(dv_o, 1)

        @block.tensor
        def _(pe: bass.BassTensorEngine):
            ci = 0
            for w in range(WPC):
                # gathered data ready for this window (all 8 streams)
                for q in range(4):
                    jn = ((w + 1) * K[q] - 1) // CHUNKS_PER_INSTR
                    for s in range(2):
                        pe.wait_ge(gsem[q][s], 16 * (jn + 1))
                if w >= 2:
                    pe.wait_ge(act_a, w - 1)  # psum_agg[w%2] drained
                first = True
                nmm = 0
                total_mm = 2 * SUMK
                for q in range(4):
                    for k in range(K[q]):
                        pos = w * K[q] + k
                        j = pos // CHUNKS_PER_INSTR
                        kk = pos % CHUNKS_PER_INSTR
                        pe.wait_ge(evsem, ci + 1)
                        for s in range(2):
                            mm = pe.matmul(
                                ps_agg[w % 2][:, 0:P],
                                lhsT=ep_t[ci % NEBUF][:],
                                rhs=dst[q][s][j % 2][:, kk, :],
                                start=first,
                                stop=(nmm == total_mm - 1),
                            )
                            if s == 1:
                                mm.then_inc(pe_c, 1)
                            first = False
                            nmm += 1
                        ci += 1
                # epilogue: transpose + GEMM
                pe.wait_ge(act_a, w + 1)
                pe.transpose(ps_tr[w % 2][:, 0:P], agg_t[w % 2][:],
                             ident_t[:]).then_inc(pe_t, 1)
                pe.wait_ge(act_b, w + 1)
                if w >= 2:
                    pe.wait_ge(dv_o, w - 1)  # ps_gm[w%2] consumed
                pe.matmul(
                    ps_gm[w % 2][:, 0:P],
                    lhsT=aggT_t[w % 2][:],
                    rhs=wt_t[:],
                    start=True,
                    stop=True,
                ).then_inc(pe_g, 1)

        @block.scalar
        def _(ac: bass.BassScalarEngine):
            for w in range(WPC):
                ac.wait_ge(pe_c, (w + 1) * SUMK)
                if w >= 2:
                    ac.wait_ge(pe_t, w - 1)  # agg_t[w%2] consumed by transpose
                ac.activation(agg_t[w % 2][:], ps_agg[w % 2][:, 0:P],
                              mybir.ActivationFunctionType.Copy).then_inc(act_a, 1)
                ac.wait_ge(pe_t, w + 1)
                if w >= 2:
                    ac.wait_ge(pe_g, w - 1)
                ac.activation(aggT_t[w % 2][:], ps_tr[w % 2][:, 0:P],
                              mybir.ActivationFunctionType.Copy).then_inc(act_b, 1)
                ac.wait_ge(dv_o, w + 1)
                if w >= 1:
                    ac.wait_ge(osem, 16 * w)
                ac.dma_start(out_d[w], out_t[w % 2][:]).then_inc(osem, 16)
            ac.wait_ge(osem, 16 * WPC)

    nc.compile()
    meta = dict(instrs=instrs, idx_cols=idx_cols, IDXCOLS=IDXCOLS, Lq=Lq,
                n_chunks=n_chunks)
    return nc, meta


def _host_arrays(plan, meta, gi16, gj16, rel, wv, features, weight, bias):
    K = plan["K"]
    CAPC = plan["CAPC"]
    n_chunks = meta["n_chunks"]
    IDXCOLS = meta["IDXCOLS"]
    instrs = meta["instrs"]
    idx_cols = meta["idx_cols"]
    base_q = plan["base_q"]

    tab = np.ascontiguousarray(features.reshape(N_NODES // 2, 256)).astype(ml_dtypes.bfloat16)
    iota = np.tile(np.arange(P, dtype=np.float32), (P, 1))
    ident = np.eye(P, dtype=np.float32)
    wt = weight.astype(np.float32)
    biasb = np.tile(bias.astype(np.float32), (P, 1))

    per_core = []
    for c in range(NCORES):
        idx_arr = np.zeros((P, IDXCOLS), dtype=np.int16)
        for q in range(4):
            for s in range(2):
                src = (gi16 if s == 0 else gj16)[c]
                for j, (soff, ni) in enumerate(instrs[q]):
                    seg = src[base_q[q] + soff : base_q[q] + soff + ni]
                    co = idx_cols[q][s][j]
                    idx_arr[:, co : co + ni // 16] = _wrap16_block(seg)
        rel_arr = np.ascontiguousarray(rel[c].reshape(n_chunks, P).T)
        wv_arr = np.ascontiguousarray(wv[c].reshape(n_chunks, P).T)
        per_core.append({
            "tab": tab, "idx": idx_arr, "rel": rel_arr, "wv": wv_arr,
            "iota": iota, "ident": ident, "wt": wt, "biasb": biasb,
        })
    return per_core


def kernel(features, pair_w, weight, bias, pair_i, pair_j, node_idx):
    features = np.asarray(features, dtype=np.float32)
    pair_w = np.asarray(pair_w, dtype=np.float32)
    weight = np.asarray(weight, dtype=np.float32)
    bias = np.asarray(bias, dtype=np.float32)
    pair_i = np.asarray(pair_i).astype(np.int64)
    pair_j = np.asarray(pair_j).astype(np.int64)
    node_idx_i = np.asarray(node_idx).astype(np.int64)

    plan, gi16, gj16, rel, wv = _plan(node_idx_i, pair_i, pair_j, pair_w)
    nc, meta = _build(plan)
    in_maps = _host_arrays(plan, meta, gi16, gj16, rel, wv, features, weight, bias)
    res = run_bass_kernel_spmd(nc, in_maps, list(range(NCORES)))
    outs = [np.asarray(res.results[c]["out"], dtype=np.float32).reshape(WPC * P, P)
            for c in range(NCORES)]
    full = np.concatenate(outs, axis=0)[:N_NODES]
    return full.astype(np.float32)


# expose for test.py profiling
def kernel_profiled(features, pair_w, weight, bias, pair_i, pair_j, node_idx):
    features = np.asarray(features, dtype=np.float32)
    pair_w = np.asarray(pair_w, dtype=np.float32)
    weight = np.asarray(weight, dtype=np.float32)
    bias = np.asarray(bias, dtype=np.float32)
    pair_i = np.asarray(pair_i).astype(np.int64)
    pair_j = np.asarray(pair_j).astype(np.int64)
    node_idx_i = np.asarray(node_idx).astype(np.int64)

    plan, gi16, gj16, rel, wv = _plan(node_idx_i, pair_i, pair_j, pair_w)
    nc, meta = _build(plan)
    in_maps = _host_arrays(plan, meta, gi16, gj16, rel, wv, features, weight, bias)
    res = run_bass_kernel_spmd(nc, in_maps, list(range(NCORES)), trace=True)
    outs = [np.asarray(res.results[c]["out"], dtype=np.float32).reshape(WPC * P, P)
            for c in range(NCORES)]
    full = np.concatenate(outs, axis=0)[:N_NODES]
    return full.astype(np.float32), res


# revision 7
# speedup vs baseline: 2.8969x; 2.8969x over previous
"""GNN message passing + aggregation + linear projection on 8 TRN2 NeuronCores.

Reference computation:
    msgs = (features[pair_i] + features[pair_j]) * pair_w[:, None]   # [P, 128]
    agg  = segment_sum(msgs, node_idx, 50000)                        # [N, 128]
    out  = agg @ weight + bias                                       # [N, 128]

Strategy (edge parallelism, no collectives):
  - node space padded to 50176 = 8 cores x 49 windows x 128 nodes.
  - node_idx is sorted, so pairs are sharded by center-node window; each core
    owns a contiguous disjoint node range => no cross-core reduction needed.
  - per (window, class) groups: class = (i%2, j%2) parity; pairs are gathered
    with dma_gather (int16 indices) from a [25000, 256] bf16 parity view of
    the feature table (node n -> row n>>1, column half n&1).
  - per 128-pair chunk: DVE builds E' [c,n] = (iota_n == rel_c) * w_c (one-hot
    times weight); PE accumulates psum[n,d] += E'.T @ gathered_i + E'.T @
    gathered_j over the window; then transpose + GEMM with weight + bias.
"""
import numpy as np
import ml_dtypes
import contextlib

import concourse.bass as bass
import concourse.mybir as mybir
from concourse import bacc
from concourse.bass_utils import run_bass_kernel_spmd
from concourse.library_config import mlp

P = 128
N_NODES = 50000
NPAD = 50176            # 392 windows * 128
NWIN = 392              # global windows
NCORES = 8
WPC = NWIN // NCORES    # 49 windows per core
D = 128
NI_MAX = 4096           # idxs per dma_gather instruction
CHUNKS_PER_INSTR = NI_MAX // P  # 32


def _wrap16_block(v):
    """[ni] int16 -> [128, ni//16] wrapped-16 replicated layout."""
    ni = v.shape[0]
    a = v.reshape(ni // 16, 16).T          # [16, ni//16]
    return np.tile(a, (8, 1))              # [128, ni//16]


def _plan(node_idx, pair_i, pair_j, pair_w):
    n = node_idx.shape[0]
    win = (node_idx // P).astype(np.int64)
    cls = ((pair_i & 1) * 2 + (pair_j & 1)).astype(np.int64)
    key = win * 4 + cls
    perm = np.argsort(key, kind="stable")
    skey = key[perm]
    counts = np.bincount(key, minlength=NWIN * 4).reshape(NWIN, 4)
    K = np.maximum(1, np.ceil(counts.max(axis=0) / P).astype(np.int64))  # per class
    SUMK = int(K.sum())
    CAPC = WPC * SUMK * P  # slots per core
    # class-major slot bases within a core
    base_q = np.zeros(4, dtype=np.int64)
    for q in range(1, 4):
        base_q[q] = base_q[q - 1] + WPC * K[q - 1] * P

    group_starts = np.searchsorted(skey, np.arange(NWIN * 4), side="left")
    rank = np.arange(n) - group_starts[skey]
    w_s = win[perm]
    q_s = cls[perm]
    core_s = w_s // WPC
    slot_in_core = base_q[q_s] + (w_s % WPC) * K[q_s] * P + rank
    gslot = core_s * CAPC + slot_in_core

    tot = NCORES * CAPC
    gi16 = np.zeros(tot, dtype=np.int16)
    gj16 = np.zeros(tot, dtype=np.int16)
    icol = np.zeros(tot, dtype=np.int8)   # parity of i (for sanity only)
    rel = np.full(tot, -1.0, dtype=np.float32)
    wv = np.zeros(tot, dtype=np.float32)

    gi16[gslot] = (pair_i[perm] >> 1).astype(np.int16)
    gj16[gslot] = (pair_j[perm] >> 1).astype(np.int16)
    icol[gslot] = (pair_i[perm] & 1).astype(np.int8)
    rel[gslot] = (node_idx[perm] - w_s * P).astype(np.float32)
    wv[gslot] = pair_w[perm].astype(np.float32)

    # dummy slots in class q must gather from the right parity column; idx 0 is
    # always valid for either column, so nothing else needed.
    plan = dict(K=[int(k) for k in K], SUMK=SUMK, CAPC=CAPC, base_q=base_q)
    return plan, gi16.reshape(NCORES, CAPC), gj16.reshape(NCORES, CAPC), \
        rel.reshape(NCORES, CAPC), wv.reshape(NCORES, CAPC)


def _instr_list(Lq):
    """Split a class stream of Lq idxs into dma_gather instruction sizes."""
    out = []
    off = 0
    while off < Lq:
        ni = min(NI_MAX, Lq - off)
        out.append((off, ni))
        off += ni
    return out


def _build(plan):
    K = plan["K"]
    SUMK = plan["SUMK"]
    base_q = plan["base_q"]
    # per class stream length (both sides identical)
    Lq = [WPC * K[q] * P for q in range(4)]
    instrs = [_instr_list(Lq[q]) for q in range(4)]   # same for both sides
    n_chunks = WPC * SUMK                              # per core chunk count
    # idx dram col offsets per (q, s, j)
    idx_cols = [[[0] * len(instrs[q]) for _ in range(2)] for q in range(4)]
    off = 0
    for q in range(4):
        for s in range(2):
            for j, (soff, ni) in enumerate(instrs[q]):
                idx_cols[q][s][j] = off
                off += ni // 16
    IDXCOLS = off

    nc = bacc.Bacc(num_swdge_queues=4)
    dt = mybir.dt
    tab = nc.declare_dram_parameter("tab", [N_NODES // 2, 256], dt.bfloat16, isOutput=False)
    idx_d = nc.declare_dram_parameter("idx", [P, IDXCOLS], dt.int16, isOutput=False)
    rel_d = nc.declare_dram_parameter("rel", [P, n_chunks], dt.float32, isOutput=False)
    wv_d = nc.declare_dram_parameter("wv", [P, n_chunks], dt.float32, isOutput=False)
    iota_d = nc.declare_dram_parameter("iota", [P, P], dt.float32, isOutput=False)
    ident_d = nc.declare_dram_parameter("ident", [P, P], dt.float32, isOutput=False)
    wt_d = nc.declare_dram_parameter("wt", [P, P], dt.float32, isOutput=False)
    bias_d = nc.declare_dram_parameter("biasb", [P, P], dt.float32, isOutput=False)
    out_d = nc.declare_dram_parameter("out", [WPC, P, P], dt.float32, isOutput=True)

    NEBUF = 6

    with (
        nc.Block() as block,
        contextlib.ExitStack() as st,
    ):
        sem = nc.semaphore
        gsem = [[st.enter_context(sem(f"g{q}{s}")) for s in range(2)] for q in range(4)]
        isem = [[st.enter_context(sem(f"i{q}{s}")) for s in range(2)] for q in range(4)]
        csem = st.enter_context(sem("consts"))
        evsem = st.enter_context(sem("ev"))
        pe_c = st.enter_context(sem("pe_c"))
        pe_t = st.enter_context(sem("pe_t"))
        pe_g = st.enter_context(sem("pe_g"))
        act_a = st.enter_context(sem("act_a"))
        act_b = st.enter_context(sem("act_b"))
        dv_o = st.enter_context(sem("dv_o"))
        osem = st.enter_context(sem("osem"))

        sb = lambda name, shape, d_: st.enter_context(nc.sbuf_tensor(name, shape, d_))
        dst = [[[sb(f"dst{q}{s}{b}", [P, CHUNKS_PER_INSTR, D], dt.bfloat16)
                 for b in range(2)] for s in range(2)] for q in range(4)]
        idx_t = [[[sb(f"idx{q}{s}{b}", [P, NI_MAX // 16], dt.int16)
                   for b in range(2)] for s in range(2)] for q in range(4)]
        rel_t = sb("rel_t", [P, n_chunks], dt.float32)
        wv_t = sb("wv_t", [P, n_chunks], dt.float32)
        iota_t = sb("iota_t", [P, P], dt.float32)
        ident_t = sb("ident_t", [P, P], dt.float32)
        wt_t = sb("wt_t", [P, P], dt.float32)
        bias_t = sb("bias_t", [P, P], dt.float32)
        ep_t = [sb(f"ep{b}", [P, P], dt.bfloat16) for b in range(NEBUF)]
        agg_t = [sb(f"agg{b}", [P, P], dt.float32) for b in range(2)]
        aggT_t = [sb(f"aggT{b}", [P, P], dt.float32) for b in range(2)]
        out_t = [sb(f"out{b}", [P, P], dt.float32) for b in range(2)]

        ps = lambda name: st.enter_context(nc.psum_tensor(name, [P, 512], dt.float32))
        ps_agg = [ps(f"psagg{b}") for b in range(2)]
        ps_tr = [ps(f"pstr{b}") for b in range(2)]
        ps_gm = [ps(f"psgm{b}") for b in range(2)]

        # round-robin gather issue order shared by sync (idx loads) and gpsimd
        issue = []  # (q, s, j, dram_coloff, ni)
        maxj = max(len(instrs[q]) for q in range(4))
        for j in range(maxj):
            for q in range(4):
                if j < len(instrs[q]):
                    for s in range(2):
                        issue.append((q, s, j, idx_cols[q][s][j], instrs[q][j][1]))

        # window consumed by end of window w -> pe_w >= w+1
        # gather (q,s,j) with j>=2 may overwrite dst buffer of instr j-2 whose
        # last chunk is in window wlast = (min((j-1)*32, Lq/128)-1)//K[q]
        def wlast(q, j):
            last_chunk = min((j - 1) * CHUNKS_PER_INSTR, Lq[q] // P) - 1
            return last_chunk // K[q]

        @block.sync
        def _(eng: bass.BassEngine):
            for ii, (dst_sb, src_d) in enumerate([
                (rel_t, rel_d), (wv_t, wv_d), (iota_t, iota_d),
                (ident_t, ident_d), (wt_t, wt_d), (bias_t, bias_d),
            ]):
                eng.dma_start(dst_sb[:], src_d[:]).then_inc(csem, 16)
                eng.wait_ge(csem, 16 * (ii + 1))
            for (q, s, j, coloff, ni) in issue:
                if j >= 1:
                    eng.wait_ge(isem[q][s], 16 * j)
                if j >= 2:
                    eng.wait_ge(gsem[q][s], 16 * (j - 1))
                eng.dma_start(
                    idx_t[q][s][j % 2][:, : ni // 16],
                    idx_d[:, coloff : coloff + ni // 16],
                ).then_inc(isem[q][s], 16)

        @block.gpsimd
        def _(gp: bass.BassGpSimd):
            gp.load_library(mlp)
            for (q, s, j, coloff, ni) in issue:
                gp.wait_ge(isem[q][s], 16 * (j + 1))
                if j >= 1:
                    gp.wait_ge(gsem[q][s], 16 * j)
                if j >= 2:
                    gp.wait_ge(pe_c, (wlast(q, j) + 1) * SUMK)
                col = (q >> 1) if s == 0 else (q & 1)
                gp.dma_gather(
                    dst[q][s][j % 2][:, : ni // P, :],
                    tab[:, col * 128 : col * 128 + 128],
                    idx_t[q][s][j % 2][:, : ni // 16],
                    ni, ni, 128,
                    elem_step=256,
                    single_packet=False,
                    queue_num=(q * 2 + s) % 4,
                ).then_inc(gsem[q][s], 16)

        @block.vector
        def _(dv: bass.BassVectorEngine):
            dv.wait_ge(csem, 96)
            ci = 0
            for w in range(WPC):
                for q in range(4):
                    for k in range(K[q]):
                        cid = base_q[q] // P + w * K[q] + k  # storage col
                        if ci >= NEBUF:
                            dv.wait_ge(pe_c, ci - NEBUF + 1)
                        dv.tensor_scalar(
                            out=ep_t[ci % NEBUF][:],
                            in0=iota_t[:],
                            scalar1=rel_t[:, cid : cid + 1],
                            scalar2=wv_t[:, cid : cid + 1],
                            op0=mybir.AluOpType.is_equal,
                            op1=mybir.AluOpType.mult,
                        ).then_inc(evsem, 1)
                        ci += 1
                # bias add for window w-1 (after its GEMM)
                if w >= 1:
                    dv.wait_ge(pe_g, w)
                    if w >= 3:
                        dv.wait_ge(osem, 16 * (w - 2))
                    dv.tensor_add(
                        out=out_t[(w - 1) % 2][:],
                        in0=ps_gm[(w - 1) % 2][:, 0:P],
                        in1=bias_t[:],
                    ).then_inc(dv_o, 1)
            dv.wait_ge(osem, 16 * (WPC - 2))
            dv.wait_ge(pe_g, WPC)
            dv.tensor_add(
                out=out_t[(WPC - 1) % 2][:],
                in0=ps_gm[(WPC - 1) % 2][:, 0:P],
                in1=bias_t[:],
            ).then_inc(dv_o, 1)

        @block.tensor
        def _(pe: bass.BassTensorEngine):
            ci = 0
            for w in range(WPC):
                # gathered data ready for this window (all 8 streams)
                for q in range(4):
                    jn = ((w + 1) * K[q] - 1) // CHUNKS_PER_INSTR
                    for s in range(2):
                        pe.wait_ge(gsem[q][s], 16 * (jn + 1))
                if w >= 2:
                    pe.wait_ge(act_a, w - 1)  # psum_agg[w%2] drained
                first = True
                nmm = 0
                total_mm = 2 * SUMK
                for q in range(4):
                    for k in range(K[q]):
                        pos = w * K[q] + k
                        j = pos // CHUNKS_PER_INSTR
                        kk = pos % CHUNKS_PER_INSTR
                        pe.wait_ge(evsem, ci + 1)
                        for s in range(2):
                            mm = pe.matmul(
                                ps_agg[w % 2][:, 0:P],
                                lhsT=ep_t[ci % NEBUF][:],
                                rhs=dst[q][s][j % 2][:, kk, :],
                                start=first,
                                stop=(nmm == total_mm - 1),
                            )
                            if s == 1:
                                mm.then_inc(pe_c, 1)
                            first = False
                            nmm += 1
                        ci += 1
                # epilogue: transpose + GEMM
                pe.wait_ge(act_a, w + 1)
                pe.transpose(ps_tr[w % 2][:, 0:P], agg_t[w % 2][:],
                             ident_t[:]).then_inc(pe_t, 1)
                pe.wait_ge(act_b, w + 1)
                if w >= 2:
                    pe.wait_ge(dv_o, w - 1)  # ps_gm[w%2] consumed
                pe.matmul(
                    ps_gm[w % 2][:, 0:P],
                    lhsT=aggT_t[w % 2][:],
                    rhs=wt_t[:],
                    start=True,
                    stop=True,
                ).then_inc(pe_g, 1)

        @block.scalar
        def _(ac: bass.BassScalarEngine):
            for w in range(WPC):
                ac.wait_ge(pe_c, (w + 1) * SUMK)
                if w >= 2:
                    ac.wait_ge(pe_t, w - 1)  # agg_t[w%2] consumed by transpose
                ac.activation(agg_t[w % 2][:], ps_agg[w % 2][:, 0:P],
                              mybir.ActivationFunctionType.Copy).then_inc(act_a, 1)
                ac.wait_ge(pe_t, w + 1)
                if w >= 2:
                    ac.wait_ge(pe_g, w - 1)
                ac.activation(aggT_t[w % 2][:], ps_tr[w % 2][:, 0:P],
                              mybir.ActivationFunctionType.Copy).then_inc(act_b, 1)
                ac.wait_ge(dv_o, w + 1)
                if w >= 1:
                    ac.wait_ge(osem, 16 * w)
                ac.dma_start(out_d[w], out_t[w % 2][:]).then_inc(osem, 16)
            ac.wait_ge(osem, 16 * WPC)

    nc.compile()
    meta = dict(instrs=instrs, idx_cols=idx_cols, IDXCOLS=IDXCOLS, Lq=Lq,
                n_chunks=n_chunks)
    return nc, meta


def _host_arrays(plan, meta, gi16, gj16, rel, wv, features, weight, bias):
    K = plan["K"]
    CAPC = plan["CAPC"]
    n_chunks = meta["n_chunks"]
    IDXCOLS = meta["IDXCOLS"]
    instrs = meta["instrs"]
    idx_cols = meta["idx_cols"]
    base_q = plan["base_q"]

    tab = np.ascontiguousarray(features.reshape(N_NODES // 2, 256)).astype(ml_dtypes.bfloat16)
    iota = np.tile(np.arange(P, dtype=np.float32), (P, 1))
    ident = np.eye(P, dtype=np.float32)
    wt = weight.astype(np.float32)
    biasb = np.tile(bias.astype(np.float32), (P, 1))

    per_core = []
    for c in range(NCORES):
        idx_arr = np.zeros((P, IDXCOLS), dtype=np.int16)
        for q in range(4):
            for s in range(2):
                src = (gi16 if s == 0 else gj16)[c]
                for j, (soff, ni) in enumerate(instrs[q]):
                    seg = src[base_q[q] + soff : base_q[q] + soff + ni]
                    co = idx_cols[q][s][j]
                    idx_arr[:, co : co + ni // 16] = _wrap16_block(seg)
        rel_arr = np.ascontiguousarray(rel[c].reshape(n_chunks, P).T)
        wv_arr = np.ascontiguousarray(wv[c].reshape(n_chunks, P).T)
        per_core.append({
            "tab": tab, "idx": idx_arr, "rel": rel_arr, "wv": wv_arr,
            "iota": iota, "ident": ident, "wt": wt, "biasb": biasb,
        })
    return per_core


def kernel(features, pair_w, weight, bias, pair_i, pair_j, node_idx):
    features = np.asarray(features, dtype=np.float32)
    pair_w = np.asarray(pair_w, dtype=np.float32)
    weight = np.asarray(weight, dtype=np.float32)
    bias = np.asarray(bias, dtype=np.float32)
    pair_i = np.asarray(pair_i).astype(np.int64)
    pair_j = np.asarray(pair_j).astype(np.int64)
    node_idx_i = np.asarray(node_idx).astype(np.int64)

    plan, gi16, gj16, rel, wv = _plan(node_idx_i, pair_i, pair_j, pair_w)
    nc, meta = _build(plan)
    in_maps = _host_arrays(plan, meta, gi16, gj16, rel, wv, features, weight, bias)
    res = run_bass_kernel_spmd(nc, in_maps, list(range(NCORES)))
    outs = [np.asarray(res.results[c]["out"], dtype=np.float32).reshape(WPC * P, P)
            for c in range(NCORES)]
    full = np.concatenate(outs, axis=0)[:N_NODES]
    return full.astype(np.float32)


# expose for test.py profiling
def kernel_profiled(features, pair_w, weight, bias, pair_i, pair_j, node_idx):
    features = np.asarray(features, dtype=np.float32)
    pair_w = np.asarray(pair_w, dtype=np.float32)
    weight = np.asarray(weight, dtype=np.float32)
    bias = np.asarray(bias, dtype=np.float32)
    pair_i = np.asarray(pair_i).astype(np.int64)
    pair_j = np.asarray(pair_j).astype(np.int64)
    node_idx_i = np.asarray(node_idx).astype(np.int64)

    plan, gi16, gj16, rel, wv = _plan(node_idx_i, pair_i, pair_j, pair_w)
    nc, meta = _build(plan)
    in_maps = _host_arrays(plan, meta, gi16, gj16, rel, wv, features, weight, bias)
    res = run_bass_kernel_spmd(nc, in_maps, list(range(NCORES)), trace=True)
    outs = [np.asarray(res.results[c]["out"], dtype=np.float32).reshape(WPC * P, P)
            for c in range(NCORES)]
    full = np.concatenate(outs, axis=0)[:N_NODES]
    return full.astype(np.float32), res


# revision 11
# speedup vs baseline: 3.0413x; 1.0498x over previous
"""GNN message passing + aggregation + linear projection on 8 TRN2 NeuronCores.

Reference computation:
    msgs = (features[pair_i] + features[pair_j]) * pair_w[:, None]   # [P, 128]
    agg  = segment_sum(msgs, node_idx, 50000)                        # [N, 128]
    out  = agg @ weight + bias                                       # [N, 128]

Strategy (edge parallelism, no collectives):
  - node space padded to 50176 = 8 cores x 49 windows x 128 nodes.
  - node_idx is sorted, so pairs are sharded by center-node window; each core
    owns a contiguous disjoint node range => no cross-core reduction needed.
  - per (window, class) groups: class = (i%2, j%2) parity; pairs are gathered
    with dma_gather (int16 indices) from a [25000, 256] bf16 parity view of
    the feature table (node n -> row n>>1, column half n&1).
  - per 128-pair chunk: DVE builds E' [c,n] = (iota_n == rel_c) * w_c (one-hot
    times weight); PE accumulates psum[n,d] += E'.T @ gathered_i + E'.T @
    gathered_j over the window; then transpose + GEMM with weight + bias.
"""
import numpy as np
import ml_dtypes
import contextlib

import concourse.bass as bass
import concourse.mybir as mybir
from concourse import bacc
from concourse.bass_utils import run_bass_kernel_spmd
from concourse.library_config import mlp

P = 128
N_NODES = 50000
NPAD = 50176            # 392 windows * 128
NWIN = 392              # global windows
NCORES = 8
WPC = NWIN // NCORES    # 49 windows per core
D = 128
NI_MAX = 4096           # idxs per dma_gather instruction
CHUNKS_PER_INSTR = NI_MAX // P  # 32


def _wrap16_block(v):
    """[ni] int16 -> [128, ni//16] wrapped-16 replicated layout."""
    ni = v.shape[0]
    a = v.reshape(ni // 16, 16).T          # [16, ni//16]
    return np.tile(a, (8, 1))              # [128, ni//16]


def _plan(node_idx, pair_i, pair_j, pair_w):
    n = node_idx.shape[0]
    win = (node_idx // P).astype(np.int64)
    cls = ((pair_i & 1) * 2 + (pair_j & 1)).astype(np.int64)
    key = win * 4 + cls
    perm = np.argsort(key, kind="stable")
    skey = key[perm]
    counts = np.bincount(key, minlength=NWIN * 4).reshape(NWIN, 4)
    K = np.maximum(1, np.ceil(counts.max(axis=0) / P).astype(np.int64))  # per class
    SUMK = int(K.sum())
    CAPC = WPC * SUMK * P  # slots per core
    # class-major slot bases within a core
    base_q = np.zeros(4, dtype=np.int64)
    for q in range(1, 4):
        base_q[q] = base_q[q - 1] + WPC * K[q - 1] * P

    group_starts = np.searchsorted(skey, np.arange(NWIN * 4), side="left")
    rank = np.arange(n) - group_starts[skey]
    w_s = win[perm]
    q_s = cls[perm]
    core_s = w_s // WPC
    slot_in_core = base_q[q_s] + (w_s % WPC) * K[q_s] * P + rank
    gslot = core_s * CAPC + slot_in_core

    tot = NCORES * CAPC
    gi16 = np.zeros(tot, dtype=np.int16)
    gj16 = np.zeros(tot, dtype=np.int16)
    icol = np.zeros(tot, dtype=np.int8)   # parity of i (for sanity only)
    rel = np.full(tot, -1.0, dtype=np.float32)
    wv = np.zeros(tot, dtype=np.float32)

    gi16[gslot] = (pair_i[perm] >> 1).astype(np.int16)
    gj16[gslot] = (pair_j[perm] >> 1).astype(np.int16)
    icol[gslot] = (pair_i[perm] & 1).astype(np.int8)
    rel[gslot] = (node_idx[perm] - w_s * P).astype(np.float32)
    wv[gslot] = pair_w[perm].astype(np.float32)

    # dummy slots in class q must gather from the right parity column; idx 0 is
    # always valid for either column, so nothing else needed.
    plan = dict(K=[int(k) for k in K], SUMK=SUMK, CAPC=CAPC, base_q=base_q)
    return plan, gi16.reshape(NCORES, CAPC), gj16.reshape(NCORES, CAPC), \
        rel.reshape(NCORES, CAPC), wv.reshape(NCORES, CAPC)


def _instr_list(Lq):
    """Split a class stream of Lq idxs into dma_gather instruction sizes."""
    out = []
    off = 0
    while off < Lq:
        ni = min(NI_MAX, Lq - off)
        out.append((off, ni))
        off += ni
    return out


def _build(plan):
    K = plan["K"]
    SUMK = plan["SUMK"]
    base_q = plan["base_q"]
    # per class stream length (both sides identical)
    Lq = [WPC * K[q] * P for q in range(4)]
    instrs = [_instr_list(Lq[q]) for q in range(4)]   # same for both sides
    n_chunks = WPC * SUMK                              # per core chunk count
    # idx dram col offsets per (q, s, j)
    idx_cols = [[[0] * len(instrs[q]) for _ in range(2)] for q in range(4)]
    off = 0
    for q in range(4):
        for s in range(2):
            for j, (soff, ni) in enumerate(instrs[q]):
                idx_cols[q][s][j] = off
                off += ni // 16
    IDXCOLS = off

    nc = bacc.Bacc(num_swdge_queues=4)
    dt = mybir.dt
    tab = nc.declare_dram_parameter("tab", [N_NODES // 2, 256], dt.bfloat16, isOutput=False)
    idx_d = nc.declare_dram_parameter("idx", [P, IDXCOLS], dt.int16, isOutput=False)
    rel_d = nc.declare_dram_parameter("rel", [P, n_chunks], dt.float32, isOutput=False)
    wv_d = nc.declare_dram_parameter("wv", [P, n_chunks], dt.float32, isOutput=False)
    iota_d = nc.declare_dram_parameter("iota", [P, P], dt.bfloat16, isOutput=False)
    ident_d = nc.declare_dram_parameter("ident", [P, P], dt.float32, isOutput=False)
    wt_d = nc.declare_dram_parameter("wt", [P, P], dt.float32, isOutput=False)
    bias_d = nc.declare_dram_parameter("biasb", [P, P], dt.float32, isOutput=False)
    out_d = nc.declare_dram_parameter("out", [WPC, P, P], dt.float32, isOutput=True)

    NEBUF = 6

    with (
        nc.Block() as block,
        contextlib.ExitStack() as st,
    ):
        sem = nc.semaphore
        gsem = [[st.enter_context(sem(f"g{q}{s}")) for s in range(2)] for q in range(4)]
        isem = [[st.enter_context(sem(f"i{q}{s}")) for s in range(2)] for q in range(4)]
        csem = st.enter_context(sem("consts"))
        evsem = st.enter_context(sem("ev"))
        pe_c = st.enter_context(sem("pe_c"))
        pe_t = st.enter_context(sem("pe_t"))
        pe_g = st.enter_context(sem("pe_g"))
        act_a = st.enter_context(sem("act_a"))
        act_b = st.enter_context(sem("act_b"))
        dv_o = st.enter_context(sem("dv_o"))
        osem = st.enter_context(sem("osem"))

        sb = lambda name, shape, d_: st.enter_context(nc.sbuf_tensor(name, shape, d_))
        dst = [[[sb(f"dst{q}{s}{b}", [P, CHUNKS_PER_INSTR, D], dt.bfloat16)
                 for b in range(2)] for s in range(2)] for q in range(4)]
        idx_t = [[[sb(f"idx{q}{s}{b}", [P, NI_MAX // 16], dt.int16)
                   for b in range(2)] for s in range(2)] for q in range(4)]
        rel_t = sb("rel_t", [P, n_chunks], dt.float32)
        wv_t = sb("wv_t", [P, n_chunks], dt.float32)
        iota_t = sb("iota_t", [P, P], dt.bfloat16)
        ident_t = sb("ident_t", [P, P], dt.float32)
        wt_t = sb("wt_t", [P, P], dt.float32)
        bias_t = sb("bias_t", [P, P], dt.float32)
        ep_t = [sb(f"ep{b}", [P, P], dt.bfloat16) for b in range(NEBUF)]
        agg_t = [sb(f"agg{b}", [P, P], dt.float32) for b in range(2)]
        aggT_t = [sb(f"aggT{b}", [P, P], dt.float32) for b in range(2)]
        out_t = [sb(f"out{b}", [P, P], dt.float32) for b in range(2)]

        ps = lambda name: st.enter_context(nc.psum_tensor(name, [P, 512], dt.float32))
        ps_agg = [ps(f"psagg{b}") for b in range(2)]
        ps_tr = [ps(f"pstr{b}") for b in range(2)]
        ps_gm = [ps(f"psgm{b}") for b in range(2)]

        # round-robin gather issue order shared by sync (idx loads) and gpsimd
        issue = []  # (q, s, j, dram_coloff, ni)
        maxj = max(len(instrs[q]) for q in range(4))
        for j in range(maxj):
            for q in range(4):
                if j < len(instrs[q]):
                    for s in range(2):
                        issue.append((q, s, j, idx_cols[q][s][j], instrs[q][j][1]))

        # window consumed by end of window w -> pe_w >= w+1
        # gather (q,s,j) with j>=2 may overwrite dst buffer of instr j-2 whose
        # last chunk is in window wlast = (min((j-1)*32, Lq/128)-1)//K[q]
        def wlast(q, j):
            last_chunk = min((j - 1) * CHUNKS_PER_INSTR, Lq[q] // P) - 1
            return last_chunk // K[q]

        @block.sync
        def _(eng: bass.BassEngine):
            for ii, (dst_sb, src_d) in enumerate([
                (rel_t, rel_d), (wv_t, wv_d), (iota_t, iota_d),
                (ident_t, ident_d), (wt_t, wt_d), (bias_t, bias_d),
            ]):
                eng.dma_start(dst_sb[:], src_d[:]).then_inc(csem, 16)
                eng.wait_ge(csem, 16 * (ii + 1))
            for (q, s, j, coloff, ni) in issue:
                if j >= 1:
                    eng.wait_ge(isem[q][s], 16 * j)
                if j >= 2:
                    eng.wait_ge(gsem[q][s], 16 * (j - 1))
                eng.dma_start(
                    idx_t[q][s][j % 2][:, : ni // 16],
                    idx_d[:, coloff : coloff + ni // 16],
                ).then_inc(isem[q][s], 16)

        @block.gpsimd
        def _(gp: bass.BassGpSimd):
            gp.load_library(mlp)
            for (q, s, j, coloff, ni) in issue:
                gp.wait_ge(isem[q][s], 16 * (j + 1))
                if j >= 1:
                    gp.wait_ge(gsem[q][s], 16 * j)
                if j >= 2:
                    gp.wait_ge(pe_c, (wlast(q, j) + 1) * SUMK)
                col = (q >> 1) if s == 0 else (q & 1)
                gp.dma_gather(
                    dst[q][s][j % 2][:, : ni // P, :],
                    tab[:, col * 128 : col * 128 + 128],
                    idx_t[q][s][j % 2][:, : ni // 16],
                    ni, ni, 128,
                    elem_step=256,
                    single_packet=False,
                    queue_num=(q * 2 + s) % 4,
                ).then_inc(gsem[q][s], 16)

        @block.vector
        def _(dv: bass.BassVectorEngine):
            dv.wait_ge(csem, 96)

            def bias_add(w):
                dv.wait_ge(pe_g, w + 1)
                if w >= 2:
                    dv.wait_ge(osem, 16 * (w - 1))  # out_t[w%2] stored for w-2
                dv.tensor_add(
                    out=out_t[w % 2][:],
                    in0=ps_gm[w % 2][:, 0:P],
                    in1=bias_t[:],
                ).then_inc(dv_o, 1)

            ci = 0
            for w in range(WPC):
                for q in range(4):
                    for k in range(K[q]):
                        cid = base_q[q] // P + w * K[q] + k  # storage col
                        if ci >= NEBUF:
                            dv.wait_ge(pe_c, ci - NEBUF + 1)
                        dv.tensor_scalar(
                            out=ep_t[ci % NEBUF][:],
                            in0=iota_t[:],
                            scalar1=rel_t[:, cid : cid + 1],
                            scalar2=wv_t[:, cid : cid + 1],
                            op0=mybir.AluOpType.is_equal,
                            op1=mybir.AluOpType.mult,
                        ).then_inc(evsem, 1)
                        ci += 1
                if w >= 2:
                    bias_add(w - 2)
            bias_add(WPC - 2)
            bias_add(WPC - 1)

        @block.tensor
        def _(pe: bass.BassTensorEngine):
            ci = 0

            def epilogue_t(w):
                # transpose of window w (agg_t[w] written by ACT copy_a(w))
                pe.wait_ge(act_a, w + 1)
                if w >= 2:
                    pe.wait_ge(act_b, w - 1)  # ps_tr[w%2] drained by copy_b(w-2)
                pe.transpose(ps_tr[w % 2][:, 0:P], agg_t[w % 2][:],
                             ident_t[:]).then_inc(pe_t, 1)

            def epilogue_g(w):
                # GEMM of window w (aggT_t[w] written by ACT copy_b(w))
                pe.wait_ge(act_b, w + 1)
                if w >= 2:
                    pe.wait_ge(dv_o, w - 1)  # ps_gm[w%2] consumed by bias-add(w-2)
                pe.matmul(
                    ps_gm[w % 2][:, 0:P],
                    lhsT=aggT_t[w % 2][:],
                    rhs=wt_t[:],
                    start=True,
                    stop=True,
                ).then_inc(pe_g, 1)

            for w in range(WPC):
                for q in range(4):
                    jn = ((w + 1) * K[q] - 1) // CHUNKS_PER_INSTR
                    for s in range(2):
                        pe.wait_ge(gsem[q][s], 16 * (jn + 1))
                if w >= 2:
                    pe.wait_ge(act_a, w - 1)  # ps_agg[w%2] drained
                first = True
                nmm = 0
                total_mm = 2 * SUMK
                for q in range(4):
                    for k in range(K[q]):
                        pos = w * K[q] + k
                        j = pos // CHUNKS_PER_INSTR
                        kk = pos % CHUNKS_PER_INSTR
                        pe.wait_ge(evsem, ci + 1)
                        for s in range(2):
                            mm = pe.matmul(
                                ps_agg[w % 2][:, 0:P],
                                lhsT=ep_t[ci % NEBUF][:],
                                rhs=dst[q][s][j % 2][:, kk, :],
                                start=first,
                                stop=(nmm == total_mm - 1),
                            )
                            if s == 1:
                                mm.then_inc(pe_c, 1)
                            first = False
                            nmm += 1
                        ci += 1
                if w >= 1:
                    epilogue_t(w - 1)
                if w >= 2:
                    epilogue_g(w - 2)
            epilogue_t(WPC - 1)
            epilogue_g(WPC - 2)
            epilogue_g(WPC - 1)

        @block.scalar
        def _(ac: bass.BassScalarEngine):
            def copy_a(w):
                ac.wait_ge(pe_c, (w + 1) * SUMK)
                if w >= 2:
                    ac.wait_ge(pe_t, w - 1)  # agg_t[w%2] consumed by transpose(w-2)
                ac.activation(agg_t[w % 2][:], ps_agg[w % 2][:, 0:P],
                              mybir.ActivationFunctionType.Copy).then_inc(act_a, 1)

            def copy_b(w):
                ac.wait_ge(pe_t, w + 1)
                if w >= 2:
                    ac.wait_ge(pe_g, w - 1)  # aggT_t[w%2] consumed by GEMM(w-2)
                ac.activation(aggT_t[w % 2][:], ps_tr[w % 2][:, 0:P],
                              mybir.ActivationFunctionType.Copy).then_inc(act_b, 1)

            def store(w):
                ac.wait_ge(dv_o, w + 1)
                if w >= 1:
                    ac.wait_ge(osem, 16 * w)
                ac.dma_start(out_d[w], out_t[w % 2][:]).then_inc(osem, 16)

            for w in range(WPC):
                copy_a(w)
                if w >= 1:
                    copy_b(w - 1)
                if w >= 2:
                    store(w - 2)
            copy_b(WPC - 1)
            store(WPC - 2)
            store(WPC - 1)
            ac.wait_ge(osem, 16 * WPC)

    nc.compile()
    meta = dict(instrs=instrs, idx_cols=idx_cols, IDXCOLS=IDXCOLS, Lq=Lq,
                n_chunks=n_chunks)
    return nc, meta


def _host_arrays(plan, meta, gi16, gj16, rel, wv, features, weight, bias):
    K = plan["K"]
    CAPC = plan["CAPC"]
    n_chunks = meta["n_chunks"]
    IDXCOLS = meta["IDXCOLS"]
    instrs = meta["instrs"]
    idx_cols = meta["idx_cols"]
    base_q = plan["base_q"]

    tab = np.ascontiguousarray(features.reshape(N_NODES // 2, 256)).astype(ml_dtypes.bfloat16)
    iota = np.tile(np.arange(P, dtype=np.float32), (P, 1)).astype(ml_dtypes.bfloat16)
    ident = np.eye(P, dtype=np.float32)
    wt = weight.astype(np.float32)
    biasb = np.tile(bias.astype(np.float32), (P, 1))

    per_core = []
    for c in range(NCORES):
        idx_arr = np.zeros((P, IDXCOLS), dtype=np.int16)
        for q in range(4):
            for s in range(2):
                src = (gi16 if s == 0 else gj16)[c]
                for j, (soff, ni) in enumerate(instrs[q]):
                    seg = src[base_q[q] + soff : base_q[q] + soff + ni]
                    co = idx_cols[q][s][j]
                    idx_arr[:, co : co + ni // 16] = _wrap16_block(seg)
        rel_arr = np.ascontiguousarray(rel[c].reshape(n_chunks, P).T)
        wv_arr = np.ascontiguousarray(wv[c].reshape(n_chunks, P).T)
        per_core.append({
            "tab": tab, "idx": idx_arr, "rel": rel_arr, "wv": wv_arr,
            "iota": iota, "ident": ident, "wt": wt, "biasb": biasb,
        })
    return per_core


def kernel(features, pair_w, weight, bias, pair_i, pair_j, node_idx):
    features = np.asarray(features, dtype=np.float32)
    pair_w = np.asarray(pair_w, dtype=np.float32)
    weight = np.asarray(weight, dtype=np.float32)
    bias = np.asarray(bias, dtype=np.float32)
    pair_i = np.asarray(pair_i).astype(np.int64)
    pair_j = np.asarray(pair_j).astype(np.int64)
    node_idx_i = np.asarray(node_idx).astype(np.int64)

    plan, gi16, gj16, rel, wv = _plan(node_idx_i, pair_i, pair_j, pair_w)
    nc, meta = _build(plan)
    in_maps = _host_arrays(plan, meta, gi16, gj16, rel, wv, features, weight, bias)
    res = run_bass_kernel_spmd(nc, in_maps, list(range(NCORES)))
    outs = [np.asarray(res.results[c]["out"], dtype=np.float32).reshape(WPC * P, P)
            for c in range(NCORES)]
    full = np.concatenate(outs, axis=0)[:N_NODES]
    return full.astype(np.float32)


# expose for test.py profiling
def kernel_profiled(features, pair_w, weight, bias, pair_i, pair_j, node_idx):
    features = np.asarray(features, dtype=np.float32)
    pair_w = np.asarray(pair_w, dtype=np.float32)
    weight = np.asarray(weight, dtype=np.float32)
    bias = np.asarray(bias, dtype=np.float32)
    pair_i = np.asarray(pair_i).astype(np.int64)
    pair_j = np.asarray(pair_j).astype(np.int64)
    node_idx_i = np.asarray(node_idx).astype(np.int64)

    plan, gi16, gj16, rel, wv = _plan(node_idx_i, pair_i, pair_j, pair_w)
    nc, meta = _build(plan)
    in_maps = _host_arrays(plan, meta, gi16, gj16, rel, wv, features, weight, bias)
    res = run_bass_kernel_spmd(nc, in_maps, list(range(NCORES)), trace=True)
    outs = [np.asarray(res.results[c]["out"], dtype=np.float32).reshape(WPC * P, P)
            for c in range(NCORES)]
    full = np.concatenate(outs, axis=0)[:N_NODES]
    return full.astype(np.float32), res


# revision 12
# speedup vs baseline: 3.2957x; 1.0836x over previous
"""GNN message passing + aggregation + linear projection on 8 TRN2 NeuronCores.

Reference computation:
    msgs = (features[pair_i] + features[pair_j]) * pair_w[:, None]   # [P, 128]
    agg  = segment_sum(msgs, node_idx, 50000)                        # [N, 128]
    out  = agg @ weight + bias                                       # [N, 128]

Strategy (edge parallelism, no collectives):
  - node space padded to 50176 = 8 cores x 49 windows x 128 nodes.
  - node_idx is sorted, so pairs are sharded by center-node window; each core
    owns a contiguous disjoint node range => no cross-core reduction needed.
  - per (window, class) groups: class = (i%2, j%2) parity; pairs are gathered
    with dma_gather (int16 indices) from a [25000, 256] bf16 parity view of
    the feature table (node n -> row n>>1, column half n&1).
  - per 128-pair chunk: DVE builds E' [c,n] = (iota_n == rel_c) * w_c (one-hot
    times weight); PE accumulates psum[n,d] += E'.T @ gathered_i + E'.T @
    gathered_j over the window; then transpose + GEMM with weight + bias.
"""
import numpy as np
import ml_dtypes
import contextlib

import concourse.bass as bass
import concourse.mybir as mybir
from concourse import bacc
from concourse.bass_utils import run_bass_kernel_spmd
from concourse.library_config import mlp

P = 128
N_NODES = 50000
NPAD = 50176            # 392 windows * 128
NWIN = 392              # global windows
NCORES = 8
WPC = NWIN // NCORES    # 49 windows per core
D = 128
NI_MAX = 2048           # idxs per dma_gather instruction
CHUNKS_PER_INSTR = NI_MAX // P
RD = 4                  # gather ring depth per stream


def _wrap16_block(v):
    """[ni] int16 -> [128, ni//16] wrapped-16 replicated layout."""
    ni = v.shape[0]
    a = v.reshape(ni // 16, 16).T          # [16, ni//16]
    return np.tile(a, (8, 1))              # [128, ni//16]


def _plan(node_idx, pair_i, pair_j, pair_w):
    n = node_idx.shape[0]
    win = (node_idx // P).astype(np.int64)
    cls = ((pair_i & 1) * 2 + (pair_j & 1)).astype(np.int64)
    key = win * 4 + cls
    perm = np.argsort(key, kind="stable")
    skey = key[perm]
    counts = np.bincount(key, minlength=NWIN * 4).reshape(NWIN, 4)
    K = np.maximum(1, np.ceil(counts.max(axis=0) / P).astype(np.int64))  # per class
    SUMK = int(K.sum())
    CAPC = WPC * SUMK * P  # slots per core
    # class-major slot bases within a core
    base_q = np.zeros(4, dtype=np.int64)
    for q in range(1, 4):
        base_q[q] = base_q[q - 1] + WPC * K[q - 1] * P

    group_starts = np.searchsorted(skey, np.arange(NWIN * 4), side="left")
    rank = np.arange(n) - group_starts[skey]
    w_s = win[perm]
    q_s = cls[perm]
    core_s = w_s // WPC
    slot_in_core = base_q[q_s] + (w_s % WPC) * K[q_s] * P + rank
    gslot = core_s * CAPC + slot_in_core

    tot = NCORES * CAPC
    gi16 = np.zeros(tot, dtype=np.int16)
    gj16 = np.zeros(tot, dtype=np.int16)
    icol = np.zeros(tot, dtype=np.int8)   # parity of i (for sanity only)
    rel = np.full(tot, -1.0, dtype=np.float32)
    wv = np.zeros(tot, dtype=np.float32)

    gi16[gslot] = (pair_i[perm] >> 1).astype(np.int16)
    gj16[gslot] = (pair_j[perm] >> 1).astype(np.int16)
    icol[gslot] = (pair_i[perm] & 1).astype(np.int8)
    rel[gslot] = (node_idx[perm] - w_s * P).astype(np.float32)
    wv[gslot] = pair_w[perm].astype(np.float32)

    # dummy slots in class q must gather from the right parity column; idx 0 is
    # always valid for either column, so nothing else needed.
    plan = dict(K=[int(k) for k in K], SUMK=SUMK, CAPC=CAPC, base_q=base_q)
    return plan, gi16.reshape(NCORES, CAPC), gj16.reshape(NCORES, CAPC), \
        rel.reshape(NCORES, CAPC), wv.reshape(NCORES, CAPC)


def _instr_list(Lq):
    """Split a class stream of Lq idxs into dma_gather instruction sizes."""
    out = []
    off = 0
    while off < Lq:
        ni = min(NI_MAX, Lq - off)
        out.append((off, ni))
        off += ni
    return out


def _build(plan):
    K = plan["K"]
    SUMK = plan["SUMK"]
    base_q = plan["base_q"]
    # per class stream length (both sides identical)
    Lq = [WPC * K[q] * P for q in range(4)]
    instrs = [_instr_list(Lq[q]) for q in range(4)]   # same for both sides
    n_chunks = WPC * SUMK                              # per core chunk count
    # idx dram col offsets per (q, s, j)
    idx_cols = [[[0] * len(instrs[q]) for _ in range(2)] for q in range(4)]
    off = 0
    for q in range(4):
        for s in range(2):
            for j, (soff, ni) in enumerate(instrs[q]):
                idx_cols[q][s][j] = off
                off += ni // 16
    IDXCOLS = off

    nc = bacc.Bacc(num_swdge_queues=4)
    dt = mybir.dt
    tab = nc.declare_dram_parameter("tab", [N_NODES // 2, 256], dt.bfloat16, isOutput=False)
    idx_d = nc.declare_dram_parameter("idx", [P, IDXCOLS], dt.int16, isOutput=False)
    rel_d = nc.declare_dram_parameter("rel", [P, n_chunks], dt.float32, isOutput=False)
    wv_d = nc.declare_dram_parameter("wv", [P, n_chunks], dt.float32, isOutput=False)
    iota_d = nc.declare_dram_parameter("iota", [P, P], dt.bfloat16, isOutput=False)
    ident_d = nc.declare_dram_parameter("ident", [P, P], dt.float32, isOutput=False)
    wt_d = nc.declare_dram_parameter("wt", [P, P], dt.float32, isOutput=False)
    bias_d = nc.declare_dram_parameter("biasb", [P, P], dt.float32, isOutput=False)
    out_d = nc.declare_dram_parameter("out", [WPC, P, P], dt.float32, isOutput=True)

    NEBUF = 8

    with (
        nc.Block() as block,
        contextlib.ExitStack() as st,
    ):
        sem = nc.semaphore
        gsem = [[st.enter_context(sem(f"g{q}{s}")) for s in range(2)] for q in range(4)]
        isem = [[st.enter_context(sem(f"i{q}{s}")) for s in range(2)] for q in range(4)]
        csem = st.enter_context(sem("consts"))
        evsem = st.enter_context(sem("ev"))
        pe_c = st.enter_context(sem("pe_c"))
        pe_t = st.enter_context(sem("pe_t"))
        pe_g = st.enter_context(sem("pe_g"))
        act_a = st.enter_context(sem("act_a"))
        act_b = st.enter_context(sem("act_b"))
        dv_o = st.enter_context(sem("dv_o"))
        osem = st.enter_context(sem("osem"))

        sb = lambda name, shape, d_: st.enter_context(nc.sbuf_tensor(name, shape, d_))
        dst = [[[sb(f"dst{q}{s}{b}", [P, CHUNKS_PER_INSTR, D], dt.bfloat16)
                 for b in range(RD)] for s in range(2)] for q in range(4)]
        idx_t = [[[sb(f"idx{q}{s}{b}", [P, NI_MAX // 16], dt.int16)
                   for b in range(RD)] for s in range(2)] for q in range(4)]
        rel_t = sb("rel_t", [P, n_chunks], dt.float32)
        wv_t = sb("wv_t", [P, n_chunks], dt.float32)
        iota_t = sb("iota_t", [P, P], dt.bfloat16)
        ident_t = sb("ident_t", [P, P], dt.float32)
        wt_t = sb("wt_t", [P, P], dt.float32)
        bias_t = sb("bias_t", [P, P], dt.float32)
        ep_t = [sb(f"ep{b}", [P, P], dt.bfloat16) for b in range(NEBUF)]
        agg_t = [sb(f"agg{b}", [P, P], dt.float32) for b in range(2)]
        aggT_t = [sb(f"aggT{b}", [P, P], dt.float32) for b in range(2)]
        out_t = [sb(f"out{b}", [P, P], dt.float32) for b in range(2)]

        ps = lambda name: st.enter_context(nc.psum_tensor(name, [P, 512], dt.float32))
        ps_agg = [ps(f"psagg{b}") for b in range(2)]
        ps_tr = [ps(f"pstr{b}") for b in range(2)]
        ps_gm = [ps(f"psgm{b}") for b in range(2)]

        # round-robin gather issue order shared by sync (idx loads) and gpsimd
        issue = []  # (q, s, j, dram_coloff, ni)
        maxj = max(len(instrs[q]) for q in range(4))
        for j in range(maxj):
            for q in range(4):
                if j < len(instrs[q]):
                    for s in range(2):
                        issue.append((q, s, j, idx_cols[q][s][j], instrs[q][j][1]))

        # window consumed by end of window w -> pe_w >= w+1
        # gather (q,s,j) with j>=2 may overwrite dst buffer of instr j-2 whose
        # last chunk is in window wlast = (min((j-1)*32, Lq/128)-1)//K[q]
        def wlast(q, j):
            last_chunk = min((j - RD + 1) * CHUNKS_PER_INSTR, Lq[q] // P) - 1
            return last_chunk // K[q]

        @block.sync
        def _(eng: bass.BassEngine):
            for ii, (dst_sb, src_d) in enumerate([
                (rel_t, rel_d), (wv_t, wv_d), (iota_t, iota_d),
                (ident_t, ident_d), (wt_t, wt_d), (bias_t, bias_d),
            ]):
                eng.dma_start(dst_sb[:], src_d[:]).then_inc(csem, 16)
                eng.wait_ge(csem, 16 * (ii + 1))
            for (q, s, j, coloff, ni) in issue:
                if j >= 1:
                    eng.wait_ge(isem[q][s], 16 * j)
                if j >= RD:
                    eng.wait_ge(gsem[q][s], 16 * (j - RD + 1))
                eng.dma_start(
                    idx_t[q][s][j % RD][:, : ni // 16],
                    idx_d[:, coloff : coloff + ni // 16],
                ).then_inc(isem[q][s], 16)

        @block.gpsimd
        def _(gp: bass.BassGpSimd):
            gp.load_library(mlp)
            for (q, s, j, coloff, ni) in issue:
                gp.wait_ge(isem[q][s], 16 * (j + 1))
                if j >= 1:
                    gp.wait_ge(gsem[q][s], 16 * j)
                if j >= RD:
                    gp.wait_ge(pe_c, (wlast(q, j) + 1) * SUMK)
                col = (q >> 1) if s == 0 else (q & 1)
                gp.dma_gather(
                    dst[q][s][j % RD][:, : ni // P, :],
                    tab[:, col * 128 : col * 128 + 128],
                    idx_t[q][s][j % RD][:, : ni // 16],
                    ni, ni, 128,
                    elem_step=256,
                    single_packet=False,
                    queue_num=(q * 2 + s) % 4,
                ).then_inc(gsem[q][s], 16)

        @block.vector
        def _(dv: bass.BassVectorEngine):
            dv.wait_ge(csem, 96)

            def bias_add(w):
                dv.wait_ge(pe_g, w + 1)
                if w >= 2:
                    dv.wait_ge(osem, 16 * (w - 1))  # out_t[w%2] stored for w-2
                dv.tensor_add(
                    out=out_t[w % 2][:],
                    in0=ps_gm[w % 2][:, 0:P],
                    in1=bias_t[:],
                ).then_inc(dv_o, 1)

            ci = 0
            for w in range(WPC):
                for q in range(4):
                    for k in range(K[q]):
                        cid = base_q[q] // P + w * K[q] + k  # storage col
                        if ci >= NEBUF:
                            dv.wait_ge(pe_c, ci - NEBUF + 1)
                        dv.tensor_scalar(
                            out=ep_t[ci % NEBUF][:],
                            in0=iota_t[:],
                            scalar1=rel_t[:, cid : cid + 1],
                            scalar2=wv_t[:, cid : cid + 1],
                            op0=mybir.AluOpType.is_equal,
                            op1=mybir.AluOpType.mult,
                        ).then_inc(evsem, 1)
                        ci += 1
                if w >= 2:
                    bias_add(w - 2)
            bias_add(WPC - 2)
            bias_add(WPC - 1)

        @block.tensor
        def _(pe: bass.BassTensorEngine):
            ci = 0

            def epilogue_t(w):
                # transpose of window w (agg_t[w] written by ACT copy_a(w))
                pe.wait_ge(act_a, w + 1)
                if w >= 2:
                    pe.wait_ge(act_b, w - 1)  # ps_tr[w%2] drained by copy_b(w-2)
                pe.transpose(ps_tr[w % 2][:, 0:P], agg_t[w % 2][:],
                             ident_t[:]).then_inc(pe_t, 1)

            def epilogue_g(w):
                # GEMM of window w (aggT_t[w] written by ACT copy_b(w))
                pe.wait_ge(act_b, w + 1)
                if w >= 2:
                    pe.wait_ge(dv_o, w - 1)  # ps_gm[w%2] consumed by bias-add(w-2)
                pe.matmul(
                    ps_gm[w % 2][:, 0:P],
                    lhsT=aggT_t[w % 2][:],
                    rhs=wt_t[:],
                    start=True,
                    stop=True,
                ).then_inc(pe_g, 1)

            for w in range(WPC):
                for q in range(4):
                    jn = ((w + 1) * K[q] - 1) // CHUNKS_PER_INSTR
                    for s in range(2):
                        pe.wait_ge(gsem[q][s], 16 * (jn + 1))
                if w >= 2:
                    pe.wait_ge(act_a, w - 1)  # ps_agg[w%2] drained
                first = True
                nmm = 0
                total_mm = 2 * SUMK
                for q in range(4):
                    for k in range(K[q]):
                        pos = w * K[q] + k
                        j = pos // CHUNKS_PER_INSTR
                        kk = pos % CHUNKS_PER_INSTR
                        pe.wait_ge(evsem, ci + 1)
                        for s in range(2):
                            mm = pe.matmul(
                                ps_agg[w % 2][:, 0:P],
                                lhsT=ep_t[ci % NEBUF][:],
                                rhs=dst[q][s][j % RD][:, kk, :],
                                start=first,
                                stop=(nmm == total_mm - 1),
                            )
                            if s == 1:
                                mm.then_inc(pe_c, 1)
                            first = False
                            nmm += 1
                        ci += 1
                if w >= 1:
                    epilogue_t(w - 1)
                if w >= 2:
                    epilogue_g(w - 2)
            epilogue_t(WPC - 1)
            epilogue_g(WPC - 2)
            epilogue_g(WPC - 1)

        @block.scalar
        def _(ac: bass.BassScalarEngine):
            def copy_a(w):
                ac.wait_ge(pe_c, (w + 1) * SUMK)
                if w >= 2:
                    ac.wait_ge(pe_t, w - 1)  # agg_t[w%2] consumed by transpose(w-2)
                ac.activation(agg_t[w % 2][:], ps_agg[w % 2][:, 0:P],
                              mybir.ActivationFunctionType.Copy).then_inc(act_a, 1)

            def copy_b(w):
                ac.wait_ge(pe_t, w + 1)
                if w >= 2:
                    ac.wait_ge(pe_g, w - 1)  # aggT_t[w%2] consumed by GEMM(w-2)
                ac.activation(aggT_t[w % 2][:], ps_tr[w % 2][:, 0:P],
                              mybir.ActivationFunctionType.Copy).then_inc(act_b, 1)

            def store(w):
                ac.wait_ge(dv_o, w + 1)
                if w >= 1:
                    ac.wait_ge(osem, 16 * w)
                ac.dma_start(out_d[w], out_t[w % 2][:]).then_inc(osem, 16)

            for w in range(WPC):
                copy_a(w)
                if w >= 1:
                    copy_b(w - 1)
                if w >= 2:
                    store(w - 2)
            copy_b(WPC - 1)
            store(WPC - 2)
            store(WPC - 1)
            ac.wait_ge(osem, 16 * WPC)

    nc.compile()
    meta = dict(instrs=instrs, idx_cols=idx_cols, IDXCOLS=IDXCOLS, Lq=Lq,
                n_chunks=n_chunks)
    return nc, meta


def _host_arrays(plan, meta, gi16, gj16, rel, wv, features, weight, bias):
    K = plan["K"]
    CAPC = plan["CAPC"]
    n_chunks = meta["n_chunks"]
    IDXCOLS = meta["IDXCOLS"]
    instrs = meta["instrs"]
    idx_cols = meta["idx_cols"]
    base_q = plan["base_q"]

    tab = np.ascontiguousarray(features.reshape(N_NODES // 2, 256)).astype(ml_dtypes.bfloat16)
    iota = np.tile(np.arange(P, dtype=np.float32), (P, 1)).astype(ml_dtypes.bfloat16)
    ident = np.eye(P, dtype=np.float32)
    wt = weight.astype(np.float32)
    biasb = np.tile(bias.astype(np.float32), (P, 1))

    per_core = []
    for c in range(NCORES):
        idx_arr = np.zeros((P, IDXCOLS), dtype=np.int16)
        for q in range(4):
            for s in range(2):
                src = (gi16 if s == 0 else gj16)[c]
                for j, (soff, ni) in enumerate(instrs[q]):
                    seg = src[base_q[q] + soff : base_q[q] + soff + ni]
                    co = idx_cols[q][s][j]
                    idx_arr[:, co : co + ni // 16] = _wrap16_block(seg)
        rel_arr = np.ascontiguousarray(rel[c].reshape(n_chunks, P).T)
        wv_arr = np.ascontiguousarray(wv[c].reshape(n_chunks, P).T)
        per_core.append({
            "tab": tab, "idx": idx_arr, "rel": rel_arr, "wv": wv_arr,
            "iota": iota, "ident": ident, "wt": wt, "biasb": biasb,
        })
    return per_core


def kernel(features, pair_w, weight, bias, pair_i, pair_j, node_idx):
    features = np.asarray(features, dtype=np.float32)
    pair_w = np.asarray(pair_w, dtype=np.float32)
    weight = np.asarray(weight, dtype=np.float32)
    bias = np.asarray(bias, dtype=np.float32)
    pair_i = np.asarray(pair_i).astype(np.int64)
    pair_j = np.asarray(pair_j).astype(np.int64)
    node_idx_i = np.asarray(node_idx).astype(np.int64)

    plan, gi16, gj16, rel, wv = _plan(node_idx_i, pair_i, pair_j, pair_w)
    nc, meta = _build(plan)
    in_maps = _host_arrays(plan, meta, gi16, gj16, rel, wv, features, weight, bias)
    res = run_bass_kernel_spmd(nc, in_maps, list(range(NCORES)))
    outs = [np.asarray(res.results[c]["out"], dtype=np.float32).reshape(WPC * P, P)
            for c in range(NCORES)]
    full = np.concatenate(outs, axis=0)[:N_NODES]
    return full.astype(np.float32)


# expose for test.py profiling
def kernel_profiled(features, pair_w, weight, bias, pair_i, pair_j, node_idx):
    features = np.asarray(features, dtype=np.float32)
    pair_w = np.asarray(pair_w, dtype=np.float32)
    weight = np.asarray(weight, dtype=np.float32)
    bias = np.asarray(bias, dtype=np.float32)
    pair_i = np.asarray(pair_i).astype(np.int64)
    pair_j = np.asarray(pair_j).astype(np.int64)
    node_idx_i = np.asarray(node_idx).astype(np.int64)

    plan, gi16, gj16, rel, wv = _plan(node_idx_i, pair_i, pair_j, pair_w)
    nc, meta = _build(plan)
    in_maps = _host_arrays(plan, meta, gi16, gj16, rel, wv, features, weight, bias)
    res = run_bass_kernel_spmd(nc, in_maps, list(range(NCORES)), trace=True)
    outs = [np.asarray(res.results[c]["out"], dtype=np.float32).reshape(WPC * P, P)
            for c in range(NCORES)]
    full = np.concatenate(outs, axis=0)[:N_NODES]
    return full.astype(np.float32), res


# revision 13
# speedup vs baseline: 3.3522x; 1.0172x over previous
"""GNN message passing + aggregation + linear projection on 8 TRN2 NeuronCores.

Reference computation:
    msgs = (features[pair_i] + features[pair_j]) * pair_w[:, None]   # [P, 128]
    agg  = segment_sum(msgs, node_idx, 50000)                        # [N, 128]
    out  = agg @ weight + bias                                       # [N, 128]

Strategy (edge parallelism, no collectives):
  - node space padded to 50176 = 8 cores x 49 windows x 128 nodes.
  - node_idx is sorted, so pairs are sharded by center-node window; each core
    owns a contiguous disjoint node range => no cross-core reduction needed.
  - per (window, class) groups: class = (i%2, j%2) parity; pairs are gathered
    with dma_gather (int16 indices) from a [25000, 256] bf16 parity view of
    the feature table (node n -> row n>>1, column half n&1).
  - per 128-pair chunk: DVE builds E' [c,n] = (iota_n == rel_c) * w_c (one-hot
    times weight); PE accumulates psum[n,d] += E'.T @ gathered_i + E'.T @
    gathered_j over the window; then transpose + GEMM with weight + bias.
"""
import numpy as np
import ml_dtypes
import contextlib

import concourse.bass as bass
import concourse.mybir as mybir
from concourse import bacc
from concourse.bass_utils import run_bass_kernel_spmd
from concourse.library_config import mlp

P = 128
N_NODES = 50000
NPAD = 50176            # 392 windows * 128
NWIN = 392              # global windows
NCORES = 8
WPC = NWIN // NCORES    # 49 windows per core
D = 128
NI_MAX = 2048           # idxs per dma_gather instruction
CHUNKS_PER_INSTR = NI_MAX // P
RD = 4                  # gather ring depth per stream


def _wrap16_block(v):
    """[ni] int16 -> [128, ni//16] wrapped-16 replicated layout."""
    ni = v.shape[0]
    a = v.reshape(ni // 16, 16).T          # [16, ni//16]
    return np.tile(a, (8, 1))              # [128, ni//16]


def _plan(node_idx, pair_i, pair_j, pair_w):
    n = node_idx.shape[0]
    win = (node_idx // P).astype(np.int64)
    cls = ((pair_i & 1) * 2 + (pair_j & 1)).astype(np.int64)
    key = win * 4 + cls
    perm = np.argsort(key, kind="stable")
    skey = key[perm]
    counts = np.bincount(key, minlength=NWIN * 4).reshape(NWIN, 4)
    K = np.maximum(1, np.ceil(counts.max(axis=0) / P).astype(np.int64))  # per class
    SUMK = int(K.sum())
    CAPC = WPC * SUMK * P  # slots per core
    # class-major slot bases within a core
    base_q = np.zeros(4, dtype=np.int64)
    for q in range(1, 4):
        base_q[q] = base_q[q - 1] + WPC * K[q - 1] * P

    group_starts = np.searchsorted(skey, np.arange(NWIN * 4), side="left")
    rank = np.arange(n) - group_starts[skey]
    w_s = win[perm]
    q_s = cls[perm]
    core_s = w_s // WPC
    slot_in_core = base_q[q_s] + (w_s % WPC) * K[q_s] * P + rank
    gslot = core_s * CAPC + slot_in_core

    tot = NCORES * CAPC
    gi16 = np.zeros(tot, dtype=np.int16)
    gj16 = np.zeros(tot, dtype=np.int16)
    icol = np.zeros(tot, dtype=np.int8)   # parity of i (for sanity only)
    rel = np.full(tot, -1.0, dtype=np.float32)
    wv = np.zeros(tot, dtype=np.float32)

    gi16[gslot] = (pair_i[perm] >> 1).astype(np.int16)
    gj16[gslot] = (pair_j[perm] >> 1).astype(np.int16)
    icol[gslot] = (pair_i[perm] & 1).astype(np.int8)
    rel[gslot] = (node_idx[perm] - w_s * P).astype(np.float32)
    wv[gslot] = pair_w[perm].astype(np.float32)

    # dummy slots in class q must gather from the right parity column; idx 0 is
    # always valid for either column, so nothing else needed.
    plan = dict(K=[int(k) for k in K], SUMK=SUMK, CAPC=CAPC, base_q=base_q)
    return plan, gi16.reshape(NCORES, CAPC), gj16.reshape(NCORES, CAPC), \
        rel.reshape(NCORES, CAPC), wv.reshape(NCORES, CAPC)


def _instr_list(Lq):
    """Split a class stream of Lq idxs into dma_gather instruction sizes."""
    out = []
    off = 0
    while off < Lq:
        ni = min(NI_MAX, Lq - off)
        out.append((off, ni))
        off += ni
    return out


def _build(plan):
    K = plan["K"]
    SUMK = plan["SUMK"]
    base_q = plan["base_q"]
    # per class stream length (both sides identical)
    Lq = [WPC * K[q] * P for q in range(4)]
    instrs = [_instr_list(Lq[q]) for q in range(4)]   # same for both sides
    n_chunks = WPC * SUMK                              # per core chunk count
    # idx dram col offsets per (q, s, j)
    idx_cols = [[[0] * len(instrs[q]) for _ in range(2)] for q in range(4)]
    off = 0
    for q in range(4):
        for s in range(2):
            for j, (soff, ni) in enumerate(instrs[q]):
                idx_cols[q][s][j] = off
                off += ni // 16
    IDXCOLS = off

    nc = bacc.Bacc(num_swdge_queues=4)
    dt = mybir.dt
    tab = nc.declare_dram_parameter("tab", [N_NODES // 2, 256], dt.bfloat16, isOutput=False)
    idx_d = nc.declare_dram_parameter("idx", [P, IDXCOLS], dt.int16, isOutput=False)
    rel_d = nc.declare_dram_parameter("rel", [P, n_chunks], dt.float32, isOutput=False)
    wv_d = nc.declare_dram_parameter("wv", [P, n_chunks], dt.float32, isOutput=False)
    iota_d = nc.declare_dram_parameter("iota", [P, P], dt.bfloat16, isOutput=False)
    ident_d = nc.declare_dram_parameter("ident", [P, P], dt.float32, isOutput=False)
    wt_d = nc.declare_dram_parameter("wt", [P, P], dt.float32, isOutput=False)
    bias_d = nc.declare_dram_parameter("biasb", [P, P], dt.float32, isOutput=False)
    out_d = nc.declare_dram_parameter("out", [WPC, P, P], dt.float32, isOutput=True)

    NEBUF = 8

    with (
        nc.Block() as block,
        contextlib.ExitStack() as st,
    ):
        sem = nc.semaphore
        gsem = [[st.enter_context(sem(f"g{q}{s}")) for s in range(2)] for q in range(4)]
        isem = [[st.enter_context(sem(f"i{q}{s}")) for s in range(2)] for q in range(4)]
        csem = st.enter_context(sem("consts"))
        evsem = st.enter_context(sem("ev"))
        pe_c = st.enter_context(sem("pe_c"))
        pe_t = st.enter_context(sem("pe_t"))
        pe_g = st.enter_context(sem("pe_g"))
        act_a = st.enter_context(sem("act_a"))
        act_b = st.enter_context(sem("act_b"))
        dv_o = st.enter_context(sem("dv_o"))
        osem = st.enter_context(sem("osem"))

        sb = lambda name, shape, d_: st.enter_context(nc.sbuf_tensor(name, shape, d_))
        dst = [[[sb(f"dst{q}{s}{b}", [P, CHUNKS_PER_INSTR, D], dt.bfloat16)
                 for b in range(RD)] for s in range(2)] for q in range(4)]
        idx_t = [[[sb(f"idx{q}{s}{b}", [P, NI_MAX // 16], dt.int16)
                   for b in range(RD)] for s in range(2)] for q in range(4)]
        rel_t = sb("rel_t", [P, n_chunks], dt.float32)
        wv_t = sb("wv_t", [P, n_chunks], dt.float32)
        iota_t = sb("iota_t", [P, P], dt.bfloat16)
        ident_t = sb("ident_t", [P, P], dt.float32)
        wt_t = sb("wt_t", [P, P], dt.float32)
        bias_t = sb("bias_t", [P, P], dt.float32)
        ep_t = [sb(f"ep{b}", [P, P], dt.bfloat16) for b in range(NEBUF)]
        agg_t = [sb(f"agg{b}", [P, P], dt.float32) for b in range(2)]
        aggT_t = [sb(f"aggT{b}", [P, P], dt.float32) for b in range(2)]
        out_t = [sb(f"out{b}", [P, P], dt.float32) for b in range(2)]

        ps = lambda name: st.enter_context(nc.psum_tensor(name, [P, 512], dt.float32))
        ps_agg = [ps(f"psagg{b}") for b in range(2)]
        ps_tr = [ps(f"pstr{b}") for b in range(2)]
        ps_gm = [ps(f"psgm{b}") for b in range(2)]

        # round-robin gather issue order shared by sync (idx loads) and gpsimd
        issue = []  # (q, s, j, dram_coloff, ni)
        maxj = max(len(instrs[q]) for q in range(4))
        for j in range(maxj):
            for q in range(4):
                if j < len(instrs[q]):
                    for s in range(2):
                        issue.append((q, s, j, idx_cols[q][s][j], instrs[q][j][1]))

        # window consumed by end of window w -> pe_w >= w+1
        # gather (q,s,j) with j>=2 may overwrite dst buffer of instr j-2 whose
        # last chunk is in window wlast = (min((j-1)*32, Lq/128)-1)//K[q]
        def wlast(q, j):
            last_chunk = min((j - RD + 1) * CHUNKS_PER_INSTR, Lq[q] // P) - 1
            return last_chunk // K[q]

        @block.sync
        def _(eng: bass.BassEngine):
            for (q, s, j, coloff, ni) in issue:
                if j >= 1:
                    eng.wait_ge(isem[q][s], 16 * j)
                if j >= RD:
                    eng.wait_ge(gsem[q][s], 16 * (j - RD + 1))
                eng.dma_start(
                    idx_t[q][s][j % RD][:, : ni // 16],
                    idx_d[:, coloff : coloff + ni // 16],
                ).then_inc(isem[q][s], 16)

        @block.gpsimd
        def _(gp: bass.BassGpSimd):
            gp.load_library(mlp)
            for (q, s, j, coloff, ni) in issue:
                gp.wait_ge(isem[q][s], 16 * (j + 1))
                if j >= 1:
                    gp.wait_ge(gsem[q][s], 16 * j)
                if j >= RD:
                    gp.wait_ge(pe_c, (wlast(q, j) + 1) * SUMK)
                col = (q >> 1) if s == 0 else (q & 1)
                gp.dma_gather(
                    dst[q][s][j % RD][:, : ni // P, :],
                    tab[:, col * 128 : col * 128 + 128],
                    idx_t[q][s][j % RD][:, : ni // 16],
                    ni, ni, 128,
                    elem_step=256,
                    single_packet=False,
                    queue_num=(q * 2 + s) % 4,
                ).then_inc(gsem[q][s], 16)

        @block.vector
        def _(dv: bass.BassVectorEngine):
            dv.wait_ge(csem, 96)

            def bias_add(w):
                dv.wait_ge(pe_g, w + 1)
                if w >= 2:
                    dv.wait_ge(osem, 16 * (w - 1))  # out_t[w%2] stored for w-2
                dv.tensor_add(
                    out=out_t[w % 2][:],
                    in0=ps_gm[w % 2][:, 0:P],
                    in1=bias_t[:],
                ).then_inc(dv_o, 1)

            ci = 0
            for w in range(WPC):
                for q in range(4):
                    for k in range(K[q]):
                        cid = base_q[q] // P + w * K[q] + k  # storage col
                        if ci >= NEBUF:
                            dv.wait_ge(pe_c, ci - NEBUF + 1)
                        dv.tensor_scalar(
                            out=ep_t[ci % NEBUF][:],
                            in0=iota_t[:],
                            scalar1=rel_t[:, cid : cid + 1],
                            scalar2=wv_t[:, cid : cid + 1],
                            op0=mybir.AluOpType.is_equal,
                            op1=mybir.AluOpType.mult,
                        ).then_inc(evsem, 1)
                        ci += 1
                if w >= 2:
                    bias_add(w - 2)
            bias_add(WPC - 2)
            bias_add(WPC - 1)

        @block.tensor
        def _(pe: bass.BassTensorEngine):
            ci = 0

            def epilogue_t(w):
                # transpose of window w (agg_t[w] written by ACT copy_a(w))
                pe.wait_ge(act_a, w + 1)
                if w >= 2:
                    pe.wait_ge(act_b, w - 1)  # ps_tr[w%2] drained by copy_b(w-2)
                pe.transpose(ps_tr[w % 2][:, 0:P], agg_t[w % 2][:],
                             ident_t[:]).then_inc(pe_t, 1)

            def epilogue_g(w):
                # GEMM of window w (aggT_t[w] written by ACT copy_b(w))
                pe.wait_ge(act_b, w + 1)
                if w >= 2:
                    pe.wait_ge(dv_o, w - 1)  # ps_gm[w%2] consumed by bias-add(w-2)
                pe.matmul(
                    ps_gm[w % 2][:, 0:P],
                    lhsT=aggT_t[w % 2][:],
                    rhs=wt_t[:],
                    start=True,
                    stop=True,
                ).then_inc(pe_g, 1)

            for w in range(WPC):
                for q in range(4):
                    jn = ((w + 1) * K[q] - 1) // CHUNKS_PER_INSTR
                    for s in range(2):
                        pe.wait_ge(gsem[q][s], 16 * (jn + 1))
                if w >= 2:
                    pe.wait_ge(act_a, w - 1)  # ps_agg[w%2] drained
                first = True
                nmm = 0
                total_mm = 2 * SUMK
                for q in range(4):
                    for k in range(K[q]):
                        pos = w * K[q] + k
                        j = pos // CHUNKS_PER_INSTR
                        kk = pos % CHUNKS_PER_INSTR
                        pe.wait_ge(evsem, ci + 1)
                        for s in range(2):
                            mm = pe.matmul(
                                ps_agg[w % 2][:, 0:P],
                                lhsT=ep_t[ci % NEBUF][:],
                                rhs=dst[q][s][j % RD][:, kk, :],
                                start=first,
                                stop=(nmm == total_mm - 1),
                            )
                            if s == 1:
                                mm.then_inc(pe_c, 1)
                            first = False
                            nmm += 1
                        ci += 1
                if w >= 1:
                    epilogue_t(w - 1)
                if w >= 2:
                    epilogue_g(w - 2)
            epilogue_t(WPC - 1)
            epilogue_g(WPC - 2)
            epilogue_g(WPC - 1)

        @block.scalar
        def _(ac: bass.BassScalarEngine):
            for ii, (dst_sb, src_d) in enumerate([
                (rel_t, rel_d), (wv_t, wv_d), (iota_t, iota_d),
                (ident_t, ident_d), (wt_t, wt_d), (bias_t, bias_d),
            ]):
                ac.dma_start(dst_sb[:], src_d[:]).then_inc(csem, 16)
                ac.wait_ge(csem, 16 * (ii + 1))

            def copy_a(w):
                ac.wait_ge(pe_c, (w + 1) * SUMK)
                if w >= 2:
                    ac.wait_ge(pe_t, w - 1)  # agg_t[w%2] consumed by transpose(w-2)
                ac.activation(agg_t[w % 2][:], ps_agg[w % 2][:, 0:P],
                              mybir.ActivationFunctionType.Copy).then_inc(act_a, 1)

            def copy_b(w):
                ac.wait_ge(pe_t, w + 1)
                if w >= 2:
                    ac.wait_ge(pe_g, w - 1)  # aggT_t[w%2] consumed by GEMM(w-2)
                ac.activation(aggT_t[w % 2][:], ps_tr[w % 2][:, 0:P],
                              mybir.ActivationFunctionType.Copy).then_inc(act_b, 1)

            def store(w):
                ac.wait_ge(dv_o, w + 1)
                if w >= 1:
                    ac.wait_ge(osem, 16 * w)
                ac.dma_start(out_d[w], out_t[w % 2][:]).then_inc(osem, 16)

            for w in range(WPC):
                copy_a(w)
                if w >= 1:
                    copy_b(w - 1)
                if w >= 2:
                    store(w - 2)
            copy_b(WPC - 1)
            store(WPC - 2)
            store(WPC - 1)
            ac.wait_ge(osem, 16 * WPC)

    nc.compile()
    meta = dict(instrs=instrs, idx_cols=idx_cols, IDXCOLS=IDXCOLS, Lq=Lq,
                n_chunks=n_chunks)
    return nc, meta


def _host_arrays(plan, meta, gi16, gj16, rel, wv, features, weight, bias):
    K = plan["K"]
    CAPC = plan["CAPC"]
    n_chunks = meta["n_chunks"]
    IDXCOLS = meta["IDXCOLS"]
    instrs = meta["instrs"]
    idx_cols = meta["idx_cols"]
    base_q = plan["base_q"]

    tab = np.ascontiguousarray(features.reshape(N_NODES // 2, 256)).astype(ml_dtypes.bfloat16)
    iota = np.tile(np.arange(P, dtype=np.float32), (P, 1)).astype(ml_dtypes.bfloat16)
    ident = np.eye(P, dtype=np.float32)
    wt = weight.astype(np.float32)
    biasb = np.tile(bias.astype(np.float32), (P, 1))

    per_core = []
    for c in range(NCORES):
        idx_arr = np.zeros((P, IDXCOLS), dtype=np.int16)
        for q in range(4):
            for s in range(2):
                src = (gi16 if s == 0 else gj16)[c]
                for j, (soff, ni) in enumerate(instrs[q]):
                    seg = src[base_q[q] + soff : base_q[q] + soff + ni]
                    co = idx_cols[q][s][j]
                    idx_arr[:, co : co + ni // 16] = _wrap16_block(seg)
        rel_arr = np.ascontiguousarray(rel[c].reshape(n_chunks, P).T)
        wv_arr = np.ascontiguousarray(wv[c].reshape(n_chunks, P).T)
        per_core.append({
            "tab": tab, "idx": idx_arr, "rel": rel_arr, "wv": wv_arr,
            "iota": iota, "ident": ident, "wt": wt, "biasb": biasb,
        })
    return per_core


def kernel(features, pair_w, weight, bias, pair_i, pair_j, node_idx):
    features = np.asarray(features, dtype=np.float32)
    pair_w = np.asarray(pair_w, dtype=np.float32)
    weight = np.asarray(weight, dtype=np.float32)
    bias = np.asarray(bias, dtype=np.float32)
    pair_i = np.asarray(pair_i).astype(np.int64)
    pair_j = np.asarray(pair_j).astype(np.int64)
    node_idx_i = np.asarray(node_idx).astype(np.int64)

    plan, gi16, gj16, rel, wv = _plan(node_idx_i, pair_i, pair_j, pair_w)
    nc, meta = _build(plan)
    in_maps = _host_arrays(plan, meta, gi16, gj16, rel, wv, features, weight, bias)
    res = run_bass_kernel_spmd(nc, in_maps, list(range(NCORES)))
    outs = [np.asarray(res.results[c]["out"], dtype=np.float32).reshape(WPC * P, P)
            for c in range(NCORES)]
    full = np.concatenate(outs, axis=0)[:N_NODES]
    return full.astype(np.float32)


# expose for test.py profiling
def kernel_profiled(features, pair_w, weight, bias, pair_i, pair_j, node_idx):
    features = np.asarray(features, dtype=np.float32)
    pair_w = np.asarray(pair_w, dtype=np.float32)
    weight = np.asarray(weight, dtype=np.float32)
    bias = np.asarray(bias, dtype=np.float32)
    pair_i = np.asarray(pair_i).astype(np.int64)
    pair_j = np.asarray(pair_j).astype(np.int64)
    node_idx_i = np.asarray(node_idx).astype(np.int64)

    plan, gi16, gj16, rel, wv = _plan(node_idx_i, pair_i, pair_j, pair_w)
    nc, meta = _build(plan)
    in_maps = _host_arrays(plan, meta, gi16, gj16, rel, wv, features, weight, bias)
    res = run_bass_kernel_spmd(nc, in_maps, list(range(NCORES)), trace=True)
    outs = [np.asarray(res.results[c]["out"], dtype=np.float32).reshape(WPC * P, P)
            for c in range(NCORES)]
    full = np.concatenate(outs, axis=0)[:N_NODES]
    return full.astype(np.float32), res
